# revision 1
# baseline (speedup 1.0000x reference)
"""Trainium2 kernel for nn_Dense_Q_MulIn1Out_Conv1D.

The reference "quantum conv" circuit is linear in the state vector: three
RY-rotation layers interleaved with a fixed 512x512 orthogonal entangler.
The whole circuit therefore collapses to one matrix M (512x512), and since
the encoded state has only its first 128 amplitudes nonzero, the <Z> readout
reduces to a quadratic form with a fixed symmetric 128x128 matrix A:

    out[n] = (v_n^T A v_n) / (||v_n||^2 + 1e-12)

where v_n is the (unnormalized) im2col patch of x (C=16 channels x K=8 taps,
channel-major).  A = Md^T Z Md with Md = M[:, :128], Z = diag(+1 x256, -1 x256).

Host side: build A (float64) from entangle_matrix/theta, permute it to
k-major patch order so the on-device im2col is 8 shifted row-block copies.
Device side (per core, 2 of 16 batches): build V [128, 4096] per batch by
DMA, Y = A @ V on TensorE (fp32r), P1 = V*Y, P2 = V*V elementwise, then
ones-vector matmuls reduce partitions to num/den rows of a [16, 512] PSUM
tile; final reciprocal-multiply and one 32KB store.
"""

import numpy as np

_DIM = 512
_D = 128
_K = 8
_C = 16
_NQ = 9
_B = 16
_L = 4096
_L_OUT = _L - _K + 1  # 4089
_N_CORES = 8
_B_PER_CORE = _B // _N_CORES  # 2
_NCHUNK = 8  # 512-column chunks per batch
_CHUNK = 512

# k-major patch permutation: new index p = k*16 + c  <->  old index c*8 + k
_PERM = np.array([(p % _C) * _K + (p // _C) for p in range(_D)])


def _apply_ry_layer(psi, angles):
    # psi [N, DIM] float64; matches reference._apply_ry_layer
    for q in range(_NQ):
        half = angles[q] * 0.5
        c, s = np.cos(half), np.sin(half)
        left = 2 ** q
        p = psi.reshape(-1, left, 2, _DIM // (2 ** (q + 1)))
        a, b = p[:, :, 0, :].copy(), p[:, :, 1, :].copy()
        psi = np.stack([c * a - s * b, s * a + c * b], axis=2).reshape(-1, _DIM)
    return psi


def _build_amat(entangle_matrix, theta):
    """Collapse the circuit to the k-major-permuted 128x128 quadratic form."""
    U = np.asarray(entangle_matrix, dtype=np.float64)
    th = np.asarray(theta, dtype=np.float64)
    psi = np.eye(_DIM, dtype=np.float64)
    for l in range(th.shape[0]):
        psi = _apply_ry_layer(psi, th[l])
        psi = psi @ U.T
    M = psi.T  # state map: s -> M s
    z = np.concatenate([np.ones(_DIM // 2), -np.ones(_DIM // 2)])
    Md = M[:, :_D]
    A = Md.T @ (z[:, None] * Md)
    A_km = A[np.ix_(_PERM, _PERM)]
    return np.ascontiguousarray(A_km, dtype=np.float32)


_NC_CACHE = {}


def _build_nc():
    import concourse.tile as tile
    from concourse import bacc, mybir

    F32 = mybir.dt.float32
    F32R = mybir.dt.float32r
    AF = mybir.ActivationFunctionType

    nc = bacc.Bacc(
        "TRN2",
        target_bir_lowering=False,
        debug=False,
        num_devices=_N_CORES,
    )
    ngl = _B_PER_CORE * _NCHUNK  # 16 global chunks
    # flat x + 8 pad elements so the im2col window never reads out of bounds
    x = nc.dram_tensor(
        "x", [_B_PER_CORE * _C * _L + _K], F32, kind="ExternalInput"
    ).ap()
    # consts = [A_km | T2] with T2 [128, 64]: single ones-column at col 32.
    # A 32-wide window T2[:, 32-m : 64-m] is a selector matrix whose matmul
    # sums all partitions into output partition m (ones at in-window col m).
    consts = nc.dram_tensor(
        "consts", [_D, _D + 96], F32, kind="ExternalInput"
    ).ap()
    out = nc.dram_tensor(
        "out", [_B_PER_CORE * _NCHUNK, _CHUNK], F32, kind="ExternalOutput"
    ).ap()

    with tile.TileContext(nc) as tc:
        from contextlib import ExitStack

        with ExitStack() as ctx:
            const_pool = ctx.enter_context(tc.tile_pool(name="const", bufs=1))
            v_pool = ctx.enter_context(tc.tile_pool(name="v", bufs=2))
            p_pool = ctx.enter_context(tc.tile_pool(name="p", bufs=2))
            y_pool = ctx.enter_context(tc.tile_pool(name="y", bufs=2, space="PSUM"))
            red_pool = ctx.enter_context(tc.tile_pool(name="red", bufs=1, space="PSUM"))
            o_pool = ctx.enter_context(tc.tile_pool(name="o", bufs=1))

            c_sb = const_pool.tile([_D, _D + 96], F32)
            nc.scalar.dma_start(c_sb[:].bitcast(F32R), consts[:].bitcast(F32R))
            a_sb = c_sb[:, :_D]
            t2 = c_sb[:, _D:]

            def sel_num(g):
                # ones at within-window col g -> output partition g (num)
                return t2[:, 48 - g : 96 - g].bitcast(F32R)

            def sel_den(g):
                # ones at col 32+g -> output partition 32+g (den; 32-aligned
                # so the epilogue's partition-offset reads are legal)
                return t2[:, 16 - g : 64 - g].bitcast(F32R)

            # num rows 0..15, den rows 32..47, one PSUM bank total
            red = red_pool.tile([48, _CHUNK], F32)

            from bass_rust import AP as RawAP

            # V free size is _L+1 so its partition pitch (4097) can't be
            # coalesced with the 4096-element column runs by the DMA AP
            # balancer (a flat run crossing SBUF partitions is invalid).
            _LV = _L + 1
            _Q = 1024  # quarter width: DMA piece + y-tile width
            vs = []
            for b in range(_B_PER_CORE):
                v = v_pool.tile([_D, _LV], F32, tag="v")
                vs.append(v)
                # im2col in 4 column-quarters, alternating the two HWDGE
                # rings (sync / scalar) so all 16 SDMA engines run.
                # dst partition (k*16+c), col n <- x[b, c, n+k]; cols >=
                # L_OUT pick up neighboring-channel garbage (host discards).
                for q in range(4):
                    dst = v[:, q * _Q : (q + 1) * _Q].bitcast(F32R)
                    srcap = RawAP(
                        tensor=x.tensor, offset=b * _C * _L + q * _Q,
                        ap=[[1, _K], [_L, _C], [1, _Q]],
                    ).bitcast(F32R)
                    eng = nc.sync if q % 2 == 0 else nc.scalar
                    eng.dma_start(dst, srcap)

            mm_i = 0  # running index over all 64 reduction matmuls
            for b in range(_B_PER_CORE):
                v = vs[b]
                for h in range(2):  # 2048-wide halves for the squares
                    p2 = p_pool.tile([_D, 2 * _Q], F32, tag="p2")
                    nc.scalar.activation(
                        p2[:].bitcast(F32R),
                        v[:, h * 2 * _Q : (h + 1) * 2 * _Q],
                        AF.Square,
                    )
                    for qq in range(2):  # 1024-wide y tiles
                        base = h * 2 * _Q + qq * _Q
                        g0 = b * _NCHUNK + (base // _CHUNK)
                        y = y_pool.tile([_D, _Q], F32)
                        for s in range(2):
                            nc.tensor.matmul(
                                y[:, s * _CHUNK : (s + 1) * _CHUNK],
                                a_sb.bitcast(F32R),
                                v[:, base + s * _CHUNK : base + (s + 1) * _CHUNK]
                                .bitcast(F32R),
                                start=True, stop=True,
                            )
                        p1 = p_pool.tile([_D, _Q], F32, tag="p1")
                        nc.vector.tensor_mul(
                            p1[:].bitcast(F32R), v[:, base : base + _Q], y[:]
                        )
                        for s in range(2):
                            g = g0 + s
                            sl = slice(s * _CHUNK, (s + 1) * _CHUNK)
                            nc.tensor.matmul(
                                red[:], sel_num(g), p1[:, sl].bitcast(F32R),
                                start=(mm_i == 0), stop=(mm_i == 63),
                                skip_group_check=True,
                            )
                            mm_i += 1
                            sl2 = slice(qq * _Q + s * _CHUNK,
                                        qq * _Q + (s + 1) * _CHUNK)
                            nc.tensor.matmul(
                                red[:], sel_den(g), p2[:, sl2].bitcast(F32R),
                                start=(mm_i == 0), stop=(mm_i == 63),
                                skip_group_check=True,
                            )
                            mm_i += 1

            den_sb = o_pool.tile([16, _CHUNK], F32, tag="den")
            nc.scalar.activation(den_sb[:], red[32:48, :], AF.Copy, bias=1e-12)
            rden = o_pool.tile([16, _CHUNK], F32, tag="rden")
            nc.vector.reciprocal_approx_fast(rden[:], den_sb[:])
            out_sb = o_pool.tile([16, _CHUNK], F32, tag="outsb")
            nc.vector.tensor_mul(out_sb[:], red[0:16, :], rden[:])
            nc.sync.dma_start(out[:], out_sb[:])

    nc.compile()
    return nc


def get_nc():
    if "nc" not in _NC_CACHE:
        _NC_CACHE["nc"] = _build_nc()
    return _NC_CACHE["nc"]


def kernel(x, entangle_matrix, theta, _trace=False, **trace_kwargs):
    from concourse.bass_utils import run_bass_kernel_spmd

    x = np.asarray(x, dtype=np.float32)
    amat = _build_amat(entangle_matrix, theta)
    # T2: single ones-column at col 32; sliding 32-wide windows of T2 give
    # every selector matrix (ones exactly at within-block column g).
    t2 = np.zeros((_D, 96), dtype=np.float32)
    t2[:, 48] = 1.0
    consts = np.ascontiguousarray(np.concatenate([amat, t2], axis=1))

    nc = get_nc()
    pad = np.zeros(_K, dtype=np.float32)
    in_maps = [
        {
            "x": np.concatenate(
                [x[i * _B_PER_CORE : (i + 1) * _B_PER_CORE].reshape(-1), pad]
            ),
            "consts": consts,
        }
        for i in range(_N_CORES)
    ]
    res = run_bass_kernel_spmd(
        nc, in_maps, list(range(_N_CORES)), trace=_trace, **trace_kwargs
    )
    outs = []
    for i in range(_N_CORES):
        o = np.asarray(res.results[i]["out"], dtype=np.float32)
        outs.append(o.reshape(_B_PER_CORE, _NCHUNK * _CHUNK)[:, :_L_OUT])
    full = np.concatenate(outs, axis=0).reshape(_B, 1, 1, _L_OUT)
    if _trace:
        kernel._last_results = res
    return full



# revision 11
# speedup vs baseline: 1.1372x; 1.1372x over previous
"""Trainium2 kernel for nn_Dense_Q_MulIn1Out_Conv1D.

The reference "quantum conv" circuit is linear in the state vector: three
RY-rotation layers interleaved with a fixed 512x512 orthogonal entangler.
The whole circuit therefore collapses to one matrix M (512x512), and since
the encoded state has only its first 128 amplitudes nonzero, the <Z> readout
reduces to a quadratic form with a fixed symmetric 128x128 matrix A:

    out[n] = (v_n^T A v_n) / (||v_n||^2 + 1e-12)

where v_n is the (unnormalized) im2col patch of x (C=16 channels x K=8 taps,
channel-major).  A = Md^T Z Md with Md = M[:, :128], Z = diag(+1 x256, -1 x256).

Device plan (per core, 2 of 16 batches), fp16 data path:
  - x is pre-cast to fp16 on host; im2col V [128, 4128] per batch is built by
    two large aligned DMAs (k-major patch order -> 8KB contiguous HBM runs).
  - A 20-matmul warmup burst (A@A) runs while the first DMA streams, pushing
    the PE HAM throttle to 2.4 GHz before real work arrives.
  - Per 1024-col chunk pair: Y = A @ V on TensorE (fp16, fp32 PSUM),
    P1 = V*Y on VectorE, P2 = V*V on ScalarE (both fp16, 1024-col ops).
  - Partition reduction via ones-selector matmuls, col-tiled 4 ways
    (num-even/num-odd/den-even/den-odd at PE col groups 0/32/64/96) so four
    512-col reduce matmuls run concurrently.
  - Epilogue: eps-add + reciprocal + multiply on 8-partition slices, two
    strided output DMAs.
"""

import numpy as np

_DIM = 512
_D = 128
_K = 8
_C = 16
_NQ = 9
_B = 16
_L = 4096
_L_OUT = _L - _K + 1  # 4089
_N_CORES = 8
_B_PER_CORE = _B // _N_CORES  # 2
_CHUNK = 512
_NCHUNK = 8  # 512-col chunks per batch
_LV = 4128  # V free size (64B-aligned partition pitch in fp16)
_NWARM = 28

# k-major patch permutation: new index p = k*16 + c  <->  old index c*8 + k
_PERM = np.array([(p % _C) * _K + (p // _C) for p in range(_D)])


def _apply_ry_layer(psi, angles):
    # psi [N, DIM] float64; matches reference._apply_ry_layer
    for q in range(_NQ):
        half = angles[q] * 0.5
        c, s = np.cos(half), np.sin(half)
        left = 2 ** q
        p = psi.reshape(-1, left, 2, _DIM // (2 ** (q + 1)))
        a, b = p[:, :, 0, :].copy(), p[:, :, 1, :].copy()
        psi = np.stack([c * a - s * b, s * a + c * b], axis=2).reshape(-1, _DIM)
    return psi


def _build_amat(entangle_matrix, theta):
    """Collapse the circuit to the k-major-permuted 128x128 quadratic form."""
    U = np.asarray(entangle_matrix, dtype=np.float64)
    th = np.asarray(theta, dtype=np.float64)
    psi = np.eye(_DIM, dtype=np.float64)
    for l in range(th.shape[0]):
        psi = _apply_ry_layer(psi, th[l])
        psi = psi @ U.T
    M = psi.T  # state map: s -> M s
    z = np.concatenate([np.ones(_DIM // 2), -np.ones(_DIM // 2)])
    Md = M[:, :_D]
    A = Md.T @ (z[:, None] * Md)
    A_km = A[np.ix_(_PERM, _PERM)]
    return np.ascontiguousarray(A_km)


_NC_CACHE = {}


def _build_nc(dbg=False):
    import concourse.tile as tile
    from concourse import bacc, mybir
    from bass_rust import AP as RawAP

    F16 = mybir.dt.float16
    F32 = mybir.dt.float32
    AF = mybir.ActivationFunctionType

    nc = bacc.Bacc(
        "TRN2",
        target_bir_lowering=False,
        debug=False,
        num_devices=_N_CORES,
    )
    dbg_t = (
        nc.dram_tensor("dbg", [_D, 5 * _CHUNK], F32, kind="ExternalOutput").ap()
        if dbg
        else None
    )
    # flat fp16 x for this core's 2 batches + 64 pad elements so the im2col
    # window never reads out of bounds
    x = nc.dram_tensor(
        "x", [_B_PER_CORE * _C * _L + 64], F16, kind="ExternalInput"
    ).ap()
    # consts = [A_km (128 cols) | T2 (96 cols)], T2 has a single ones-column
    # at col 48: the 32-wide window T2[:, 48-u : 80-u] is a selector whose
    # matmul sums all 128 partitions into output partition u.
    consts = nc.dram_tensor("consts", [_D, 224], F16, kind="ExternalInput").ap()
    out = nc.dram_tensor(
        "out", [2 * _NCHUNK, _CHUNK], F32, kind="ExternalOutput"
    ).ap()

    with tile.TileContext(nc) as tc:
        from contextlib import ExitStack

        with ExitStack() as ctx:
            const_pool = ctx.enter_context(tc.tile_pool(name="const", bufs=1))
            v_pool = ctx.enter_context(tc.tile_pool(name="v", bufs=2))
            p1_pool = ctx.enter_context(tc.tile_pool(name="p1", bufs=2))
            p2_pool = ctx.enter_context(tc.tile_pool(name="p2", bufs=2))
            y_pool = ctx.enter_context(tc.tile_pool(name="y", bufs=2, space="PSUM"))
            red_pool = ctx.enter_context(tc.tile_pool(name="red", bufs=1, space="PSUM"))
            warm_pool = ctx.enter_context(tc.tile_pool(name="warm", bufs=1, space="PSUM"))
            o_pool = ctx.enter_context(tc.tile_pool(name="o", bufs=1))

            c_sb = const_pool.tile([_D, 224], F16)
            nc.sync.dma_start(c_sb[:], consts[:])
            a_sb = c_sb[:, :_D]
            t2 = c_sb[:, _D:]

            def sel(g):
                # 16-wide window: ones at within-window col g -> out partition g
                return t2[:, 48 - g : 64 - g]

            # Warmup burst: keeps PE busy during the V DMAs so HAM reaches
            # 2.4 GHz before the first real matmul. Results never read.
            warm = warm_pool.tile([_D, _CHUNK], F32)
            for _ in range(_NWARM):
                nc.tensor.matmul(
                    warm[:, :_D], a_sb, a_sb, start=True, stop=True
                )

            # im2col: V[k*16+c, n] = x[b, c, n+k]; one whole-batch SWDGE DMA
            # per batch -> one 8256B descriptor per partition (HWDGE 4KB
            # descriptors measured only ~9 GB/s/engine, HBM-latency-bound).
            vs = []
            for b in range(_B_PER_CORE):
                v = v_pool.tile([_D, _LV], F16, tag="v")
                vs.append(v)
                srcap = RawAP(
                    tensor=x.tensor,
                    offset=b * _C * _L,
                    ap=[[1, _K], [_L, _C], [1, _LV]],
                )
                nc.gpsimd.dma_start(v[:], srcap)

            # red PSUM tile, 2-way col tiling:
            #   partitions [0:16]  num of chunk g at partition g
            #   partitions [32:48] den of chunk g at partition g
            red = red_pool.tile([48, _CHUNK], F32)

            pending = None  # (p1, p2, u) reduces issued after next pair's mains
            n_pairs = _B_PER_CORE * _NCHUNK // 2  # 8

            def emit_reduces(p1, p2, u):
                kw = dict(skip_group_check=True)
                for h in range(2):  # chunk halves 2u, 2u+1
                    g = 2 * u + h
                    s = sel(g)
                    sl = slice(h * _CHUNK, (h + 1) * _CHUNK)
                    nc.tensor.matmul(
                        red[0:16, :], s, p1[:, sl], tile_position=(0, 0),
                        start=(g == 0), stop=(g == 2 * n_pairs - 1), **kw
                    )
                    nc.tensor.matmul(
                        red[32:48, :], s, p2[:, sl], tile_position=(0, 32),
                        start=(g == 0), stop=(g == 2 * n_pairs - 1), **kw
                    )

            for u in range(n_pairs):
                b = u // (n_pairs // _B_PER_CORE)
                base = (u % (n_pairs // _B_PER_CORE)) * 2 * _CHUNK
                v = vs[b]
                y = y_pool.tile([_D, 2 * _CHUNK], F32)
                for s2 in range(2):
                    nc.tensor.matmul(
                        y[:, s2 * _CHUNK : (s2 + 1) * _CHUNK],
                        a_sb,
                        v[:, base + s2 * _CHUNK : base + (s2 + 1) * _CHUNK],
                        start=True,
                        stop=True,
                    )
                if pending is not None:
                    emit_reduces(*pending)
                p1 = p1_pool.tile([_D, 2 * _CHUNK], F16, tag="p1")
                nc.vector.tensor_mul(
                    p1[:], v[:, base : base + 2 * _CHUNK], y[:]
                )
                p2 = p2_pool.tile([_D, 2 * _CHUNK], F16, tag="p2")
                nc.scalar.activation(
                    p2[:], v[:, base : base + 2 * _CHUNK], AF.Square
                )
                if dbg and u == 0:
                    dbg_sb = o_pool.tile([_D, 4 * _CHUNK], F32, tag="dbg")
                    nc.scalar.activation(dbg_sb[:, :_CHUNK], v[:, :_CHUNK], AF.Copy)
                    nc.scalar.activation(
                        dbg_sb[:, _CHUNK : 2 * _CHUNK], y[:, :_CHUNK], AF.Copy
                    )
                    nc.scalar.activation(
                        dbg_sb[:, 2 * _CHUNK : 3 * _CHUNK], p1[:, :_CHUNK], AF.Copy
                    )
                    nc.scalar.activation(
                        dbg_sb[:, 3 * _CHUNK : 4 * _CHUNK], p2[:, :_CHUNK], AF.Copy
                    )
                    nc.sync.dma_start(dbg_t[:, : 4 * _CHUNK], dbg_sb[:])
                pending = (p1, p2, u)
            emit_reduces(*pending)
            if dbg:
                red_sb = o_pool.tile([48, _CHUNK], F32, tag="redsb")
                nc.scalar.activation(red_sb[:], red[:], AF.Copy)
                nc.sync.dma_start(dbg_t[:48, 4 * _CHUNK :], red_sb[:])

            # epilogue: out = num / den. den is a sum of ~128 squares of
            # N(0,1) data (~128 +- 16), so the reference's 1e-12 eps is
            # numerically irrelevant and skipped. DVE cannot read PSUM at a
            # nonzero base partition (reads silently wrap to base 0), so den
            # is staged through ScalarE, which can.
            den_sb = o_pool.tile([16, _CHUNK], F32, tag="den_sb")
            nc.scalar.activation(den_sb[:], red[32:48, :], AF.Copy)
            rden = o_pool.tile([16, _CHUNK], F32, tag="rden")
            nc.vector.reciprocal_approx_fast(rden[:], den_sb[:])
            out_sb = o_pool.tile([16, _CHUNK], F32, tag="out_sb")
            nc.vector.tensor_mul(out_sb[:], red[0:16, :], rden[:])
            nc.sync.dma_start(out[:], out_sb[:])

    nc.compile()
    return nc


def get_nc():
    if "nc" not in _NC_CACHE:
        _NC_CACHE["nc"] = _build_nc()
    return _NC_CACHE["nc"]


def kernel(x, entangle_matrix, theta, _trace=False, **trace_kwargs):
    from concourse.bass_utils import run_bass_kernel_spmd

    x16 = np.asarray(x).astype(np.float16)
    amat = _build_amat(entangle_matrix, theta)
    consts = np.zeros((_D, 224), dtype=np.float16)
    consts[:, :_D] = amat.astype(np.float16)
    consts[:, _D + 48] = 1.0  # T2 ones-column

    nc = get_nc()
    pad = np.zeros(64, dtype=np.float16)
    in_maps = [
        {
            "x": np.concatenate(
                [x16[i * _B_PER_CORE : (i + 1) * _B_PER_CORE].reshape(-1), pad]
            ),
            "consts": consts,
        }
        for i in range(_N_CORES)
    ]
    res = run_bass_kernel_spmd(
        nc, in_maps, list(range(_N_CORES)), trace=_trace, **trace_kwargs
    )
    outs = []
    for i in range(_N_CORES):
        o = np.asarray(res.results[i]["out"], dtype=np.float32)
        # row g = batch (g//8), col block (g%8)
        outs.append(o.reshape(_B_PER_CORE, _NCHUNK * _CHUNK)[:, :_L_OUT])
    full = np.concatenate(outs, axis=0).reshape(_B, 1, 1, _L_OUT)
    if _trace:
        kernel._last_results = res
    return full


# revision 15
# speedup vs baseline: 1.2404x; 1.0908x over previous
"""Trainium2 kernel for nn_Dense_Q_MulIn1Out_Conv1D.

The reference "quantum conv" circuit is linear in the state vector: three
RY-rotation layers interleaved with a fixed 512x512 orthogonal entangler.
The whole circuit therefore collapses to one matrix M (512x512), and since
the encoded state has only its first 128 amplitudes nonzero, the <Z> readout
reduces to a quadratic form with a fixed symmetric 128x128 matrix A:

    out[n] = (v_n^T A v_n) / (||v_n||^2 + 1e-12)

where v_n is the (unnormalized) im2col patch of x (C=16 channels x K=8 taps,
channel-major).  A = Md^T Z Md with Md = M[:, :128], Z = diag(+1 x256, -1 x256).

Device plan (per core, 2 of 16 batches), fp16 data path:
  - x is pre-cast to fp16 on host; im2col V [128, 4128] per batch is built by
    two large aligned DMAs (k-major patch order -> 8KB contiguous HBM runs).
  - A 20-matmul warmup burst (A@A) runs while the first DMA streams, pushing
    the PE HAM throttle to 2.4 GHz before real work arrives.
  - Per 1024-col chunk pair: Y = A @ V on TensorE (fp16, fp32 PSUM),
    P1 = V*Y on VectorE, P2 = V*V on ScalarE (both fp16, 1024-col ops).
  - Partition reduction via ones-selector matmuls, col-tiled 4 ways
    (num-even/num-odd/den-even/den-odd at PE col groups 0/32/64/96) so four
    512-col reduce matmuls run concurrently.
  - Epilogue: eps-add + reciprocal + multiply on 8-partition slices, two
    strided output DMAs.
"""

import numpy as np

_DIM = 512
_D = 128
_K = 8
_C = 16
_NQ = 9
_B = 16
_L = 4096
_L_OUT = _L - _K + 1  # 4089
_N_CORES = 8
_B_PER_CORE = _B // _N_CORES  # 2
_CHUNK = 512
_NCHUNK = 8  # 512-col chunks per batch
_LV = 4128  # V free size (64B-aligned partition pitch in fp16)
_NWARM = 18

# k-major patch permutation: new index p = k*16 + c  <->  old index c*8 + k
_PERM = np.array([(p % _C) * _K + (p // _C) for p in range(_D)])


def _apply_ry_layer(psi, angles):
    # psi [N, DIM] float64; matches reference._apply_ry_layer
    for q in range(_NQ):
        half = angles[q] * 0.5
        c, s = np.cos(half), np.sin(half)
        left = 2 ** q
        p = psi.reshape(-1, left, 2, _DIM // (2 ** (q + 1)))
        a, b = p[:, :, 0, :].copy(), p[:, :, 1, :].copy()
        psi = np.stack([c * a - s * b, s * a + c * b], axis=2).reshape(-1, _DIM)
    return psi


def _build_amat(entangle_matrix, theta):
    """Collapse the circuit to the k-major-permuted 128x128 quadratic form."""
    U = np.asarray(entangle_matrix, dtype=np.float64)
    th = np.asarray(theta, dtype=np.float64)
    psi = np.eye(_DIM, dtype=np.float64)
    for l in range(th.shape[0]):
        psi = _apply_ry_layer(psi, th[l])
        psi = psi @ U.T
    M = psi.T  # state map: s -> M s
    z = np.concatenate([np.ones(_DIM // 2), -np.ones(_DIM // 2)])
    Md = M[:, :_D]
    A = Md.T @ (z[:, None] * Md)
    A_km = A[np.ix_(_PERM, _PERM)]
    return np.ascontiguousarray(A_km)


_NC_CACHE = {}


def _build_nc(dbg=False):
    import concourse.tile as tile
    from concourse import bacc, mybir
    from bass_rust import AP as RawAP

    F16 = mybir.dt.float16
    F32 = mybir.dt.float32
    AF = mybir.ActivationFunctionType

    nc = bacc.Bacc(
        "TRN2",
        target_bir_lowering=False,
        debug=False,
        num_devices=_N_CORES,
    )
    dbg_t = (
        nc.dram_tensor("dbg", [_D, 5 * _CHUNK], F32, kind="ExternalOutput").ap()
        if dbg
        else None
    )
    # flat fp16 x for this core's 2 batches + 64 pad elements so the im2col
    # window never reads out of bounds
    x = nc.dram_tensor(
        "x", [_B_PER_CORE * _C * _L + 64], F16, kind="ExternalInput"
    ).ap()
    # consts = [A_km (128 cols) | T2 (96 cols)], T2 has a single ones-column
    # at col 48: the 32-wide window T2[:, 48-u : 80-u] is a selector whose
    # matmul sums all 128 partitions into output partition u.
    consts = nc.dram_tensor("consts", [_D, 224], F16, kind="ExternalInput").ap()
    out = nc.dram_tensor(
        "out", [2 * _NCHUNK, _CHUNK], F32, kind="ExternalOutput"
    ).ap()

    with tile.TileContext(nc) as tc:
        from contextlib import ExitStack

        with ExitStack() as ctx:
            const_pool = ctx.enter_context(tc.tile_pool(name="const", bufs=1))
            v_pool = ctx.enter_context(tc.tile_pool(name="v", bufs=2))
            p1_pool = ctx.enter_context(tc.tile_pool(name="p1", bufs=2))
            p2_pool = ctx.enter_context(tc.tile_pool(name="p2", bufs=2))
            y_pool = ctx.enter_context(tc.tile_pool(name="y", bufs=2, space="PSUM"))
            red_pool = ctx.enter_context(tc.tile_pool(name="red", bufs=1, space="PSUM"))
            warm_pool = ctx.enter_context(tc.tile_pool(name="warm", bufs=1, space="PSUM"))
            o_pool = ctx.enter_context(tc.tile_pool(name="o", bufs=1))

            # im2col first: V[k*16+c, n] = x[b, c, n+k]. Batch 0 in two
            # halves (sync + scalar HWDGE rings) so compute starts early;
            # batch 1 as one whole-batch SWDGE DMA (8256B per-partition
            # descriptors amortize the ~450ns HBM round-trip per packet).
            vs = []
            for b in range(_B_PER_CORE):
                v = v_pool.tile([_D, _LV], F16, tag="v")
                vs.append(v)
            for h, (c0, w) in enumerate(((0, 2048), (2048, _LV - 2048))):
                srcap = RawAP(
                    tensor=x.tensor, offset=c0, ap=[[1, _K], [_L, _C], [1, w]]
                )
                eng = nc.sync if h == 0 else nc.scalar
                eng.dma_start(vs[0][:, c0 : c0 + w], srcap)
            src1 = RawAP(
                tensor=x.tensor, offset=_C * _L, ap=[[1, _K], [_L, _C], [1, _LV]]
            )
            nc.gpsimd.dma_start(vs[1][:], src1)

            c_sb = const_pool.tile([_D, 224], F16)
            nc.sync.dma_start(c_sb[:], consts[:])
            a_sb = c_sb[:, :_D]
            t2 = c_sb[:, _D:]

            def sel(g):
                # 16-wide window: ones at within-window col g -> out partition g
                return t2[:, 48 - g : 64 - g]

            # Warmup burst: keeps PE busy during the V DMAs so HAM reaches
            # 2.4 GHz before the first real matmul. Results never read.
            warm = warm_pool.tile([_D, _CHUNK], F32)
            for _ in range(_NWARM):
                nc.tensor.matmul(
                    warm[:, :_D], a_sb, a_sb, start=True, stop=True
                )

            # red PSUM tile, 2-way col tiling:
            #   partitions [0:16]  num of chunk g at partition g
            #   partitions [32:48] den of chunk g at partition g
            red = red_pool.tile([48, _CHUNK], F32)

            pending = None  # (p1, p2, u) reduces issued after next pair's mains
            n_pairs = _B_PER_CORE * _NCHUNK // 2  # 8

            def emit_reduces(p1, p2, u):
                kw = dict(skip_group_check=True)
                for h in range(2):  # chunk halves 2u, 2u+1
                    g = 2 * u + h
                    s = sel(g)
                    sl = slice(h * _CHUNK, (h + 1) * _CHUNK)
                    nc.tensor.matmul(
                        red[0:16, :], s, p1[:, sl], tile_position=(0, 0),
                        start=(g == 0), stop=(g == 2 * n_pairs - 1), **kw
                    )
                    nc.tensor.matmul(
                        red[32:48, :], s, p2[:, sl], tile_position=(0, 32),
                        start=(g == 0), stop=(g == 2 * n_pairs - 1), **kw
                    )

            for u in range(n_pairs):
                b = u // (n_pairs // _B_PER_CORE)
                base = (u % (n_pairs // _B_PER_CORE)) * 2 * _CHUNK
                v = vs[b]
                # reduces of the previous pair go first: their inputs are
                # ready, so the PE does useful work during any DMA wait for
                # this pair's V columns.
                if pending is not None:
                    emit_reduces(*pending)
                    pending = None
                y = y_pool.tile([_D, 2 * _CHUNK], F32)
                for s2 in range(2):
                    nc.tensor.matmul(
                        y[:, s2 * _CHUNK : (s2 + 1) * _CHUNK],
                        a_sb,
                        v[:, base + s2 * _CHUNK : base + (s2 + 1) * _CHUNK],
                        start=True,
                        stop=True,
                    )
                p1 = p1_pool.tile([_D, 2 * _CHUNK], F16, tag="p1")
                nc.vector.tensor_mul(
                    p1[:], v[:, base : base + 2 * _CHUNK], y[:]
                )
                p2 = p2_pool.tile([_D, 2 * _CHUNK], F16, tag="p2")
                nc.scalar.activation(
                    p2[:], v[:, base : base + 2 * _CHUNK], AF.Square
                )
                if dbg and u == 0:
                    dbg_sb = o_pool.tile([_D, 4 * _CHUNK], F32, tag="dbg")
                    nc.scalar.activation(dbg_sb[:, :_CHUNK], v[:, :_CHUNK], AF.Copy)
                    nc.scalar.activation(
                        dbg_sb[:, _CHUNK : 2 * _CHUNK], y[:, :_CHUNK], AF.Copy
                    )
                    nc.scalar.activation(
                        dbg_sb[:, 2 * _CHUNK : 3 * _CHUNK], p1[:, :_CHUNK], AF.Copy
                    )
                    nc.scalar.activation(
                        dbg_sb[:, 3 * _CHUNK : 4 * _CHUNK], p2[:, :_CHUNK], AF.Copy
                    )
                    nc.sync.dma_start(dbg_t[:, : 4 * _CHUNK], dbg_sb[:])
                pending = (p1, p2, u)
            emit_reduces(*pending)
            if dbg:
                red_sb = o_pool.tile([48, _CHUNK], F32, tag="redsb")
                nc.scalar.activation(red_sb[:], red[:], AF.Copy)
                nc.sync.dma_start(dbg_t[:48, 4 * _CHUNK :], red_sb[:])

            # epilogue: out = num / den. den is a sum of ~128 squares of
            # N(0,1) data (~128 +- 16), so the reference's 1e-12 eps is
            # numerically irrelevant and skipped. DVE cannot read PSUM at a
            # nonzero base partition (reads silently wrap to base 0), so den
            # is staged through ScalarE, which can.
            den_sb = o_pool.tile([16, _CHUNK], F32, tag="den_sb")
            nc.scalar.activation(den_sb[:], red[32:48, :], AF.Copy)
            rden = o_pool.tile([16, _CHUNK], F32, tag="rden")
            nc.vector.reciprocal_approx_fast(rden[:], den_sb[:])
            out_sb = o_pool.tile([16, _CHUNK], F32, tag="out_sb")
            nc.vector.tensor_mul(out_sb[:], red[0:16, :], rden[:])
            nc.sync.dma_start(out[:], out_sb[:])

    nc.compile()
    return nc


def get_nc():
    if "nc" not in _NC_CACHE:
        _NC_CACHE["nc"] = _build_nc()
    return _NC_CACHE["nc"]


def kernel(x, entangle_matrix, theta, _trace=False, **trace_kwargs):
    from concourse.bass_utils import run_bass_kernel_spmd

    x16 = np.asarray(x).astype(np.float16)
    amat = _build_amat(entangle_matrix, theta)
    consts = np.zeros((_D, 224), dtype=np.float16)
    consts[:, :_D] = amat.astype(np.float16)
    consts[:, _D + 48] = 1.0  # T2 ones-column

    nc = get_nc()
    pad = np.zeros(64, dtype=np.float16)
    in_maps = [
        {
            "x": np.concatenate(
                [x16[i * _B_PER_CORE : (i + 1) * _B_PER_CORE].reshape(-1), pad]
            ),
            "consts": consts,
        }
        for i in range(_N_CORES)
    ]
    res = run_bass_kernel_spmd(
        nc, in_maps, list(range(_N_CORES)), trace=_trace, **trace_kwargs
    )
    outs = []
    for i in range(_N_CORES):
        o = np.asarray(res.results[i]["out"], dtype=np.float32)
        # row g = batch (g//8), col block (g%8)
        outs.append(o.reshape(_B_PER_CORE, _NCHUNK * _CHUNK)[:, :_L_OUT])
    full = np.concatenate(outs, axis=0).reshape(_B, 1, 1, _L_OUT)
    if _trace:
        kernel._last_results = res
    return full


# revision 17
# speedup vs baseline: 1.2525x; 1.0097x over previous
"""Trainium2 kernel for nn_Dense_Q_MulIn1Out_Conv1D.

The reference "quantum conv" circuit is linear in the state vector: three
RY-rotation layers interleaved with a fixed 512x512 orthogonal entangler.
The whole circuit therefore collapses to one matrix M (512x512), and since
the encoded state has only its first 128 amplitudes nonzero, the <Z> readout
reduces to a quadratic form with a fixed symmetric 128x128 matrix A:

    out[n] = (v_n^T A v_n) / (||v_n||^2 + 1e-12)

where v_n is the (unnormalized) im2col patch of x (C=16 channels x K=8 taps,
channel-major).  A = Md^T Z Md with Md = M[:, :128], Z = diag(+1 x256, -1 x256).

Device plan (per core, 2 of 16 batches), fp16 data path:
  - x is pre-cast to fp16 on host; im2col V [128, 4128] per batch is built by
    two large aligned DMAs (k-major patch order -> 8KB contiguous HBM runs).
  - A 20-matmul warmup burst (A@A) runs while the first DMA streams, pushing
    the PE HAM throttle to 2.4 GHz before real work arrives.
  - Per 1024-col chunk pair: Y = A @ V on TensorE (fp16, fp32 PSUM),
    P1 = V*Y on VectorE, P2 = V*V on ScalarE (both fp16, 1024-col ops).
  - Partition reduction via ones-selector matmuls, col-tiled 4 ways
    (num-even/num-odd/den-even/den-odd at PE col groups 0/32/64/96) so four
    512-col reduce matmuls run concurrently.
  - Epilogue: eps-add + reciprocal + multiply on 8-partition slices, two
    strided output DMAs.
"""

import numpy as np

_DIM = 512
_D = 128
_K = 8
_C = 16
_NQ = 9
_B = 16
_L = 4096
_L_OUT = _L - _K + 1  # 4089
_N_CORES = 8
_B_PER_CORE = _B // _N_CORES  # 2
_CHUNK = 512
_NCHUNK = 8  # 512-col chunks per batch
_LV = 4096  # V free size (8KB partition pitch in fp16; cols >= L_OUT hold
            # neighboring-channel garbage that the host discards)
_NWARM = 18

# k-major patch permutation: new index p = k*16 + c  <->  old index c*8 + k
_PERM = np.array([(p % _C) * _K + (p // _C) for p in range(_D)])


def _apply_ry_layer(psi, angles):
    # psi [N, DIM] float64; matches reference._apply_ry_layer
    for q in range(_NQ):
        half = angles[q] * 0.5
        c, s = np.cos(half), np.sin(half)
        left = 2 ** q
        p = psi.reshape(-1, left, 2, _DIM // (2 ** (q + 1)))
        a, b = p[:, :, 0, :].copy(), p[:, :, 1, :].copy()
        psi = np.stack([c * a - s * b, s * a + c * b], axis=2).reshape(-1, _DIM)
    return psi


def _build_amat(entangle_matrix, theta):
    """Collapse the circuit to the k-major-permuted 128x128 quadratic form."""
    U = np.asarray(entangle_matrix, dtype=np.float64)
    th = np.asarray(theta, dtype=np.float64)
    psi = np.eye(_DIM, dtype=np.float64)
    for l in range(th.shape[0]):
        psi = _apply_ry_layer(psi, th[l])
        psi = psi @ U.T
    M = psi.T  # state map: s -> M s
    z = np.concatenate([np.ones(_DIM // 2), -np.ones(_DIM // 2)])
    Md = M[:, :_D]
    A = Md.T @ (z[:, None] * Md)
    A_km = A[np.ix_(_PERM, _PERM)]
    return np.ascontiguousarray(A_km)


_NC_CACHE = {}


def _build_nc(dbg=False):
    import concourse.tile as tile
    from concourse import bacc, mybir
    from bass_rust import AP as RawAP

    F16 = mybir.dt.float16
    F32 = mybir.dt.float32
    AF = mybir.ActivationFunctionType

    nc = bacc.Bacc(
        "TRN2",
        target_bir_lowering=False,
        debug=False,
        num_devices=_N_CORES,
    )
    dbg_t = (
        nc.dram_tensor("dbg", [_D, 5 * _CHUNK], F32, kind="ExternalOutput").ap()
        if dbg
        else None
    )
    # flat fp16 x for this core's 2 batches + 64 pad elements so the im2col
    # window never reads out of bounds
    x = nc.dram_tensor(
        "x", [_B_PER_CORE * _C * _L + 64], F16, kind="ExternalInput"
    ).ap()
    # consts = [A_km (128 cols) | T2 (96 cols)], T2 has a single ones-column
    # at col 48: the 32-wide window T2[:, 48-u : 80-u] is a selector whose
    # matmul sums all 128 partitions into output partition u.
    consts = nc.dram_tensor("consts", [_D, 224], F16, kind="ExternalInput").ap()
    out = nc.dram_tensor(
        "out", [2 * _NCHUNK, _CHUNK], F32, kind="ExternalOutput"
    ).ap()

    with tile.TileContext(nc) as tc:
        from contextlib import ExitStack

        with ExitStack() as ctx:
            const_pool = ctx.enter_context(tc.tile_pool(name="const", bufs=1))
            v_pool = ctx.enter_context(tc.tile_pool(name="v", bufs=2))
            p1_pool = ctx.enter_context(tc.tile_pool(name="p1", bufs=2))
            p2_pool = ctx.enter_context(tc.tile_pool(name="p2", bufs=2))
            y_pool = ctx.enter_context(tc.tile_pool(name="y", bufs=2, space="PSUM"))
            red_pool = ctx.enter_context(tc.tile_pool(name="red", bufs=1, space="PSUM"))
            warm_pool = ctx.enter_context(tc.tile_pool(name="warm", bufs=1, space="PSUM"))
            o_pool = ctx.enter_context(tc.tile_pool(name="o", bufs=1))

            # consts go alone on the idle sync ring so warmups start early
            c_sb = const_pool.tile([_D, 224], F16)
            nc.sync.dma_start(c_sb[:], consts[:])
            a_sb = c_sb[:, :_D]
            t2 = c_sb[:, _D:]

            # im2col: V[k*16+c, n] = x[b, c, n+k]. All V pieces FIFO on the
            # gpsimd (SWDGE) ring in consumption order: one queue keeps the
            # full ~140 GB/s aggregate (multiple queues just dilute each
            # other), and 2048-col pieces let compute start ~4us in.
            vs = []
            for b in range(_B_PER_CORE):
                v = v_pool.tile([_D, _LV], F16, tag="v")
                vs.append(v)
            for b in range(_B_PER_CORE):
                for c0 in range(0, _LV, 2048):
                    srcap = RawAP(
                        tensor=x.tensor,
                        offset=b * _C * _L + c0,
                        ap=[[1, _K], [_L, _C], [1, 2048]],
                    )
                    nc.gpsimd.dma_start(vs[b][:, c0 : c0 + 2048], srcap)

            def sel(g):
                # 16-wide window: ones at within-window col g -> out partition g
                return t2[:, 48 - g : 64 - g]

            # Warmup burst: keeps PE busy during the V DMAs so HAM reaches
            # 2.4 GHz before the first real matmul. Results never read.
            warm = warm_pool.tile([_D, _CHUNK], F32)
            for _ in range(_NWARM):
                nc.tensor.matmul(
                    warm[:, :_D], a_sb, a_sb, start=True, stop=True
                )

            # red PSUM tile, 2-way col tiling:
            #   partitions [0:16]  num of chunk g at partition g
            #   partitions [32:48] den of chunk g at partition g
            red = red_pool.tile([48, _CHUNK], F32)

            pending = None  # (p1, p2, u) reduces issued after next pair's mains
            n_pairs = _B_PER_CORE * _NCHUNK // 2  # 8

            def emit_reduces(p1, p2, u):
                kw = dict(skip_group_check=True)
                for h in range(2):  # chunk halves 2u, 2u+1
                    g = 2 * u + h
                    s = sel(g)
                    sl = slice(h * _CHUNK, (h + 1) * _CHUNK)
                    nc.tensor.matmul(
                        red[0:16, :], s, p1[:, sl], tile_position=(0, 0),
                        start=(g == 0), stop=(g == 2 * n_pairs - 1), **kw
                    )
                    nc.tensor.matmul(
                        red[32:48, :], s, p2[:, sl], tile_position=(0, 32),
                        start=(g == 0), stop=(g == 2 * n_pairs - 1), **kw
                    )

            for u in range(n_pairs):
                b = u // (n_pairs // _B_PER_CORE)
                base = (u % (n_pairs // _B_PER_CORE)) * 2 * _CHUNK
                v = vs[b]
                # reduces of the previous pair go first: their inputs are
                # ready, so the PE does useful work during any DMA wait for
                # this pair's V columns.
                if pending is not None:
                    emit_reduces(*pending)
                    pending = None
                y = y_pool.tile([_D, 2 * _CHUNK], F32)
                for s2 in range(2):
                    nc.tensor.matmul(
                        y[:, s2 * _CHUNK : (s2 + 1) * _CHUNK],
                        a_sb,
                        v[:, base + s2 * _CHUNK : base + (s2 + 1) * _CHUNK],
                        start=True,
                        stop=True,
                    )
                p1 = p1_pool.tile([_D, 2 * _CHUNK], F16, tag="p1")
                nc.vector.tensor_mul(
                    p1[:], v[:, base : base + 2 * _CHUNK], y[:]
                )
                p2 = p2_pool.tile([_D, 2 * _CHUNK], F16, tag="p2")
                nc.scalar.activation(
                    p2[:], v[:, base : base + 2 * _CHUNK], AF.Square
                )
                if dbg and u == 0:
                    dbg_sb = o_pool.tile([_D, 4 * _CHUNK], F32, tag="dbg")
                    nc.scalar.activation(dbg_sb[:, :_CHUNK], v[:, :_CHUNK], AF.Copy)
                    nc.scalar.activation(
                        dbg_sb[:, _CHUNK : 2 * _CHUNK], y[:, :_CHUNK], AF.Copy
                    )
                    nc.scalar.activation(
                        dbg_sb[:, 2 * _CHUNK : 3 * _CHUNK], p1[:, :_CHUNK], AF.Copy
                    )
                    nc.scalar.activation(
                        dbg_sb[:, 3 * _CHUNK : 4 * _CHUNK], p2[:, :_CHUNK], AF.Copy
                    )
                    nc.sync.dma_start(dbg_t[:, : 4 * _CHUNK], dbg_sb[:])
                pending = (p1, p2, u)
            emit_reduces(*pending)
            if dbg:
                red_sb = o_pool.tile([48, _CHUNK], F32, tag="redsb")
                nc.scalar.activation(red_sb[:], red[:], AF.Copy)
                nc.sync.dma_start(dbg_t[:48, 4 * _CHUNK :], red_sb[:])

            # epilogue: out = num / den. den is a sum of ~128 squares of
            # N(0,1) data (~128 +- 16), so the reference's 1e-12 eps is
            # numerically irrelevant and skipped. DVE cannot read PSUM at a
            # nonzero base partition (reads silently wrap to base 0), so den
            # is staged through ScalarE, which can.
            den_sb = o_pool.tile([16, _CHUNK], F32, tag="den_sb")
            nc.scalar.activation(den_sb[:], red[32:48, :], AF.Copy)
            rden = o_pool.tile([16, _CHUNK], F32, tag="rden")
            nc.vector.reciprocal_approx_fast(rden[:], den_sb[:])
            out_sb = o_pool.tile([16, _CHUNK], F32, tag="out_sb")
            nc.vector.tensor_mul(out_sb[:], red[0:16, :], rden[:])
            nc.sync.dma_start(out[:], out_sb[:])

    nc.compile()
    return nc


def get_nc():
    if "nc" not in _NC_CACHE:
        _NC_CACHE["nc"] = _build_nc()
    return _NC_CACHE["nc"]


def kernel(x, entangle_matrix, theta, _trace=False, **trace_kwargs):
    from concourse.bass_utils import run_bass_kernel_spmd

    x16 = np.asarray(x).astype(np.float16)
    amat = _build_amat(entangle_matrix, theta)
    consts = np.zeros((_D, 224), dtype=np.float16)
    consts[:, :_D] = amat.astype(np.float16)
    consts[:, _D + 48] = 1.0  # T2 ones-column

    nc = get_nc()
    pad = np.zeros(64, dtype=np.float16)
    in_maps = [
        {
            "x": np.concatenate(
                [x16[i * _B_PER_CORE : (i + 1) * _B_PER_CORE].reshape(-1), pad]
            ),
            "consts": consts,
        }
        for i in range(_N_CORES)
    ]
    res = run_bass_kernel_spmd(
        nc, in_maps, list(range(_N_CORES)), trace=_trace, **trace_kwargs
    )
    outs = []
    for i in range(_N_CORES):
        o = np.asarray(res.results[i]["out"], dtype=np.float32)
        # row g = batch (g//8), col block (g%8)
        outs.append(o.reshape(_B_PER_CORE, _NCHUNK * _CHUNK)[:, :_L_OUT])
    full = np.concatenate(outs, axis=0).reshape(_B, 1, 1, _L_OUT)
    if _trace:
        kernel._last_results = res
    return full
